# revision 1
# baseline (speedup 1.0000x reference)
"""Trainium2 Bass kernel for nn_Backward_12094627905824 (MLP trunk + gumbel-argmax
mixture sampling). Data-parallel over 8 NeuronCores: batch B=262144 is sharded
32768 rows/core; the small MLP / head weights are replicated.

Math per batch row b (reference semantics):
  h = relu chain: 3 -> 128 -> 256 -> 200
  mu/sig/pai[g,d] = heads (25 comps x 4 dims), pai/sigma through abs
  idx[d] = argmax_g log(pai+1e-12) + gumbel[b,g,d]
  out[b,d] = rand[b,d]*|sig[idx,d]| + mu[idx,d]

On-device reformulation (argmax-invariant): score = |pai_raw| * exp(gumbel);
selection via one-hot (score >= rowmax) mask-and-sum.
"""
import numpy as np

import concourse.bass as bass
import concourse.mybir as mybir
import bass_rust
from concourse.tile import TileContext
from concourse.bass_utils import run_bass_kernel_spmd

NCORES = 8
B, G, D = 262144, 25, 4
GD = G * D                       # 100
H1, H2, H3 = 128, 256, 200
BS = B // NCORES                 # 32768 rows per core
NB = 512                         # batch columns per compute tile
NT = BS // NB                    # 64 tiles
NSUB = NB // 128                 # 4 sub-blocks of 128 rows

F32 = mybir.dt.float32
F32R = mybir.dt.float32r


def _split_multi_waits(nc):
    # walrus CoreV3 codegen accepts only one sync-wait per instruction; Tile's
    # exit drain waits once per active proc. Split into single-wait drains.
    for bb in nc.main_func.blocks:
        insts = list(bb.instructions)
        out = []
        changed = False
        for inst in insts:
            si = inst.sync_info
            if si is not None and len(si.on_wait) > 1:
                waits = list(si.on_wait)
                for k, w in enumerate(waits[:-1]):
                    d = mybir.InstDrain(name=f"{inst.name}-sw{k}", ins=[], outs=[])
                    d.engine = inst.engine
                    d.sync_info = bass_rust.SyncInfo(on_wait=[w], on_update=[])
                    nc.register_instruction(d)
                    out.append(d)
                si.on_wait = [waits[-1]]
                changed = True
            out.append(inst)
        if changed:
            bb.instructions = out


def _build_nc():
    nc = bass.Bass(trn_type="TRN2")

    x0t = nc.dram_tensor("x0t", [3, BS], F32R, kind="ExternalInput")
    gum = nc.dram_tensor("gum", [BS, GD], F32, kind="ExternalInput")
    rnd = nc.dram_tensor("rnd", [BS, D], F32, kind="ExternalInput")
    w1t = nc.dram_tensor("w1t", [3, H1], F32R, kind="ExternalInput")
    b1 = nc.dram_tensor("b1", [H1, 1], F32, kind="ExternalInput")
    w2t = nc.dram_tensor("w2t", [H1, H2], F32R, kind="ExternalInput")
    b2 = nc.dram_tensor("b2", [H2, 1], F32, kind="ExternalInput")
    w3t = nc.dram_tensor("w3t", [H2, H3], F32R, kind="ExternalInput")   # rows = h2 feat
    b3 = nc.dram_tensor("b3", [H3, 1], F32, kind="ExternalInput")
    wh = nc.dram_tensor("wh", [H3, 300], F32R, kind="ExternalInput")  # [mu|sig|pai]
    onesr = nc.dram_tensor("onesr", [2, NB], F32R, kind="ExternalInput")
    whbias = nc.dram_tensor("whbias", [2, 300], F32R, kind="ExternalInput")
    out_d = nc.dram_tensor("out", [BS, D], F32, kind="ExternalOutput")

    from contextlib import ExitStack
    with TileContext(nc) as tc, ExitStack() as ctx:
        const = ctx.enter_context(tc.tile_pool(name="const", bufs=1))
        io = ctx.enter_context(tc.tile_pool(name="io", bufs=3))
        act = ctx.enter_context(tc.tile_pool(name="act", bufs=4))
        samp = ctx.enter_context(tc.tile_pool(name="samp", bufs=3))
        ptrunk = ctx.enter_context(tc.tile_pool(name="ptrunk", bufs=3, space="PSUM"))
        pheads = ctx.enter_context(tc.tile_pool(name="pheads", bufs=4, space="PSUM"))

        # --- load weights once ---
        w1t_s = const.tile([3, H1], F32R)
        nc.sync.dma_start(out=w1t_s, in_=w1t[:, :])
        b1_s = const.tile([H1, 1], F32)
        nc.sync.dma_start(out=b1_s, in_=b1[:, :])
        w2t_s = const.tile([H1, H2], F32R)
        nc.sync.dma_start(out=w2t_s, in_=w2t[:, :])
        b2a_s = const.tile([128, 1], F32, tag="b2a")
        nc.sync.dma_start(out=b2a_s, in_=b2[0:128, :])
        b2b_s = const.tile([128, 1], F32, tag="b2b")
        nc.sync.dma_start(out=b2b_s, in_=b2[128:256, :])
        w3ta_s = const.tile([128, H3], F32R, tag="w3ta")   # h2 feats 0:128
        nc.sync.dma_start(out=w3ta_s, in_=w3t[0:128, :])
        w3tb_s = const.tile([128, H3], F32R, tag="w3tb")   # h2 feats 128:256
        nc.sync.dma_start(out=w3tb_s, in_=w3t[128:256, :])
        b3a_s = const.tile([128, 1], F32, tag="b3a")
        nc.sync.dma_start(out=b3a_s, in_=b3[0:128, :])
        b3b_s = const.tile([72, 1], F32, tag="b3b")
        nc.sync.dma_start(out=b3b_s, in_=b3[128:200, :])
        wha_s = const.tile([128, 300], F32R, tag="wha")    # h3 feats 0:128
        nc.sync.dma_start(out=wha_s, in_=wh[0:128, :])
        whb_s = const.tile([72, 300], F32R, tag="whb")     # h3 feats 128:200
        nc.sync.dma_start(out=whb_s, in_=wh[128:200, :])
        bias_row_s = const.tile([2, 300], F32R, tag="biasrow")
        nc.sync.dma_start(out=bias_row_s, in_=whbias[:, :])
        ones_s = const.tile([2, NB], F32R, tag="ones")
        nc.sync.dma_start(out=ones_s, in_=onesr[:, :])



        for it in range(NT):
            b0 = it * NB

            # --- input DMAs ---
            x_s = io.tile([3, NB], F32R, tag="x")
            nc.sync.dma_start(out=x_s, in_=x0t[:, b0:b0 + NB])
            gum_s = io.tile([128, NSUB, GD], F32, tag="gum")
            nc.sync.dma_start(
                out=gum_s,
                in_=gum[b0:b0 + NB, :].rearrange("(s p) e -> p s e", s=NSUB),
            )
            rnd_s = io.tile([128, NSUB, D], F32, tag="rnd")
            nc.sync.dma_start(
                out=rnd_s,
                in_=rnd[b0:b0 + NB, :].rearrange("(s p) d -> p s d", s=NSUB),
            )

            # --- trunk ---
            h1p = ptrunk.tile([128, NB], F32, tag="pt")
            nc.tensor.matmul(h1p, lhsT=w1t_s[:, :], rhs=x_s[:, :],
                             start=True, stop=True)
            h1 = act.tile([128, NB], F32R, tag="h1")
            nc.scalar.activation(h1, h1p, func=mybir.ActivationFunctionType.Relu,
                                 bias=b1_s[:, :], scale=1.0)

            h2ap = ptrunk.tile([128, NB], F32, tag="pt")
            nc.tensor.matmul(h2ap, lhsT=w2t_s[:, 0:128], rhs=h1[:, :],
                             start=True, stop=True)
            h2a = act.tile([128, NB], F32R, tag="h2a")
            nc.scalar.activation(h2a, h2ap, func=mybir.ActivationFunctionType.Relu,
                                 bias=b2a_s[:, :], scale=1.0)

            h2bp = ptrunk.tile([128, NB], F32, tag="pt")
            nc.tensor.matmul(h2bp, lhsT=w2t_s[:, 128:256], rhs=h1[:, :],
                             start=True, stop=True)
            h2b = act.tile([128, NB], F32R, tag="h2b")
            nc.scalar.activation(h2b, h2bp, func=mybir.ActivationFunctionType.Relu,
                                 bias=b2b_s[:, :], scale=1.0)

            h3ap = ptrunk.tile([128, NB], F32, tag="pt")
            nc.tensor.matmul(h3ap, lhsT=w3ta_s[:, 0:128], rhs=h2a[:, :],
                             start=True, stop=False)
            nc.tensor.matmul(h3ap, lhsT=w3tb_s[:, 0:128], rhs=h2b[:, :],
                             start=False, stop=True)
            h3a = act.tile([128, NB], F32R, tag="h3a")
            nc.scalar.activation(h3a, h3ap, func=mybir.ActivationFunctionType.Relu,
                                 bias=b3a_s[:, :], scale=1.0)

            h3bp = ptrunk.tile([72, NB], F32, tag="pt")
            nc.tensor.matmul(h3bp, lhsT=w3ta_s[:, 128:200], rhs=h2a[:, :],
                             start=True, stop=False)
            nc.tensor.matmul(h3bp, lhsT=w3tb_s[:, 128:200], rhs=h2b[:, :],
                             start=False, stop=True)
            h3b = act.tile([72, NB], F32R, tag="h3b")
            nc.scalar.activation(h3b, h3bp,
                                 func=mybir.ActivationFunctionType.Relu,
                                 bias=b3b_s[:, :], scale=1.0)

            # --- heads: psum[s] = [mu(100) | sig(100) | pai(100)] per 128-row sub
            hp = []
            for s in range(NSUB):
                hps = pheads.tile([128, 300], F32, tag="hp")
                c0, c1 = s * 128, (s + 1) * 128
                nc.tensor.matmul(hps, lhsT=h3a[:, c0:c1], rhs=wha_s[:, :],
                                 start=True, stop=False)
                nc.tensor.matmul(hps, lhsT=h3b[:, c0:c1], rhs=whb_s[:, :],
                                 start=False, stop=False)
                nc.tensor.matmul(hps, lhsT=ones_s[:, c0:c1], rhs=bias_row_s[:, :],
                                 start=False, stop=True)
                hp.append(hps)

            # --- sampling ---
            ex = samp.tile([128, NSUB, GD], F32, tag="ex")
            nc.scalar.activation(ex, gum_s, func=mybir.ActivationFunctionType.Exp)

            absp = samp.tile([128, NSUB, GD], F32, tag="absp")
            for s in range(NSUB):
                nc.scalar.activation(absp[:, s], hp[s][:, 200:300],
                                     func=mybir.ActivationFunctionType.Abs)
            # score = |pai_raw| * exp(gumbel)
            sc = samp.tile([128, NSUB, GD], F32, tag="sc")
            nc.vector.tensor_mul(sc, absp, ex)

            # rowmax over g per (sub, d):  view (p, s, d, g)
            sc_v = sc.rearrange("p s (g d) -> p s d g", g=G)
            smax = samp.tile([128, NSUB, D], F32, tag="smax")
            nc.vector.tensor_reduce(smax, sc_v, axis=mybir.AxisListType.X,
                                    op=mybir.AluOpType.max)

            # one-hot: oh = (score >= smax)
            oh = samp.tile([128, NSUB, GD], F32, tag="oh")
            smax_b = smax.unsqueeze(3).broadcast_to([128, NSUB, D, G])
            nc.vector.tensor_tensor(
                out=oh.rearrange("p s (g d) -> p s d g", g=G),
                in0=sc_v,
                in1=smax_b, op=mybir.AluOpType.is_ge)

            # masked select-sum of mu and sig: pms = [mu|sig] * oh
            pms = samp.tile([128, NSUB, 2, GD], F32, tag="pms")
            for s in range(NSUB):
                oh_b = oh[:, s].unsqueeze(1).broadcast_to([128, 2, GD])
                nc.vector.tensor_mul(pms[:, s], hp[s][:, 0:200]
                                     .rearrange("p (h e) -> p h e", h=2), oh_b)

            sel = samp.tile([128, NSUB, 2, D], F32, tag="sel")
            nc.vector.tensor_reduce(
                sel, pms.rearrange("p s h (g d) -> p s h d g", g=G),
                axis=mybir.AxisListType.X, op=mybir.AluOpType.add)

            # out = rnd * |sig_sel| + mu_sel
            siga = samp.tile([128, NSUB, D], F32, tag="siga")
            nc.vector.scalar_tensor_tensor(
                out=siga, in0=sel[:, :, 1, :], scalar=-1.0, in1=sel[:, :, 1, :],
                op0=mybir.AluOpType.mult, op1=mybir.AluOpType.max)
            ot = samp.tile([128, NSUB, D], F32, tag="ot")
            nc.vector.tensor_mul(ot, rnd_s, siga)
            nc.vector.tensor_add(ot, ot, sel[:, :, 0, :])

            nc.sync.dma_start(
                out=out_d[b0:b0 + NB, :].rearrange("(s p) d -> p s d", s=NSUB),
                in_=ot)

    _split_multi_waits(nc)
    return nc


_NC_CACHE = None
LAST_RESULT = None


def kernel(x0, rand, gumbel, W1, b1, W2, b2, W3, b3,
           Wmu, bmu, Wsig, bsig, Wpai, bpai):
    global _NC_CACHE, LAST_RESULT
    if _NC_CACHE is None:
        _NC_CACHE = _build_nc()
    nc = _NC_CACHE

    x0 = np.ascontiguousarray(np.asarray(x0, np.float32))
    rand = np.ascontiguousarray(np.asarray(rand, np.float32))
    gumbel = np.ascontiguousarray(np.asarray(gumbel, np.float32))

    # stacked head weights [201, 300]: rows 0..199 = h3 feats, row 200 = bias;
    # col = head*100 + g*4 + d
    WH = np.zeros((H3 + 1, 300), np.float32)
    for hd, (W, b) in enumerate([(Wmu, bmu), (Wsig, bsig), (Wpai, bpai)]):
        WH[:H3, hd * GD:(hd + 1) * GD] = np.asarray(W, np.float32).reshape(GD, H3).T
        WH[H3, hd * GD:(hd + 1) * GD] = np.asarray(b, np.float32).reshape(GD)

    def _split10(a):
        """hi = a with mantissa truncated to 10 explicit bits (exactly
        representable in fp32r), lo = exact residual."""
        a = np.ascontiguousarray(a, np.float32)
        hi = (a.view(np.uint32) & np.uint32(0xFFFFE000)).view(np.float32)
        return hi, np.ascontiguousarray(a - hi)

    w2hi = np.ascontiguousarray(np.asarray(W2, np.float32).T)
    w3hi = np.ascontiguousarray(np.asarray(W3, np.float32).T)
    whhi = np.ascontiguousarray(WH[:H3])
    bhi, blo = _split10(WH[H3:H3 + 1])
    wmats = {
        "w1t": np.ascontiguousarray(np.asarray(W1, np.float32).T),
        "b1": np.asarray(b1, np.float32).reshape(H1, 1),
        "w2t": w2hi,
        "b2": np.asarray(b2, np.float32).reshape(H2, 1),
        "w3t": w3hi,
        "b3": np.asarray(b3, np.float32).reshape(H3, 1),
        "wh": whhi,
        "whbias": np.ascontiguousarray(np.vstack([bhi, blo])),
    }

    in_maps = []
    for c in range(NCORES):
        sl = slice(c * BS, (c + 1) * BS)
        m = {
            "onesr": np.ones((2, NB), np.float32),
            "x0t": np.ascontiguousarray(x0[sl].T),
            "gum": np.ascontiguousarray(gumbel[sl].reshape(BS, GD)),
            "rnd": np.ascontiguousarray(rand[sl]),
        }
        m.update(wmats)
        in_maps.append(m)

    res = run_bass_kernel_spmd(nc, in_maps, core_ids=list(range(NCORES)))
    LAST_RESULT = res
    out = np.concatenate([res.results[c]["out"] for c in range(NCORES)], axis=0)
    return out.astype(np.float32)



# revision 22
# speedup vs baseline: 1.0330x; 1.0330x over previous
"""Trainium2 Bass kernel for nn_Backward_12094627905824 (MLP trunk + gumbel-argmax
mixture sampling). Data-parallel over 8 NeuronCores: batch B=262144 is sharded
32768 rows/core; the small MLP / head weights are replicated.

Math per batch row b (reference semantics):
  h = relu chain: 3 -> 128 -> 256 -> 200
  mu/sig/pai[g,d] = heads (25 comps x 4 dims), pai/sigma through abs
  idx[d] = argmax_g log(pai+1e-12) + gumbel[b,g,d]
  out[b,d] = rand[b,d]*|sig[idx,d]| + mu[idx,d]

On-device reformulation (argmax-invariant): score = |pai_raw| * exp(gumbel);
selection via one-hot (score >= rowmax); out = sum_g onehot * z where
z = mu + rand*|sig| is folded BEFORE the masked reduce (one select instead of
two).

v2 engine plan (per 512-row tile, 64 tiles/core):
  PE   : 13 matmuls, all biases folded in (x-pack carries no ones; trunk
         biases are added exactly in f32 by Pool/Act; head biases enter via
         two ones-rows in the h3a SBUF tile against bias hi/lo rows of the
         packed head weights - same precision as the baseline's hi/lo ones
         matmul, but zero extra PE cycles).
  Act  : h2a/h2b fused bias+relu (psum->sbuf), exp(gumbel).
  Pool : h1/h3a/h3b fused bias+relu via tensor_scalar(add bias, max 0),
         |pai|, |sig| via abs_max, z1 = |sig|*rand, z = z1 + mu.
  DVE  : score mul, rowmax reduce, one-hot is_ge, z*onehot, masked-sum.
  DMA  : everything batched: x/weights/biases/rand/out are ONE DMA each
         (host-side relayout packs them 128-partitions-tall), gumbel in 8
         contiguous group DMAs of 8 tiles each.
"""
import numpy as np

import concourse.bass as bass
import concourse.mybir as mybir
import bass_rust
from concourse.tile import TileContext
from concourse.bass_utils import run_bass_kernel_spmd

NCORES = 8
B, G, D = 262144, 25, 4
GD = G * D                       # 100
H1, H2, H3 = 128, 256, 200
BS = B // NCORES                 # 32768 rows per core
NB = 512                         # batch columns per compute tile
NT = BS // NB                    # 64 tiles
NSUB = NB // 128                 # 4 sub-blocks of 128 rows
CH = 8                           # tiles per gumbel DMA group
NGRP = NT // CH                  # 8 groups

F32 = mybir.dt.float32
F32R = mybir.dt.float32r
BF16 = mybir.dt.bfloat16

# weight-pack column map (f32r matmul operands, 128 partitions)
# W1 region: 8 shifted variants (K=32 trick): variant a has W1.T in rows
# 4a..4a+2 of a [32,128] block, zeros elsewhere; replicated at bases 0 and 64
# so lhsT.base matches the x rhs base (PE quadrant rule).
WP_W1 = 0          # [{0:32,64:96}, 0:1024]  8 x [32,128] shifted W1.T blocks
WP_W2 = 1024       # [0:128, +0:256]  W2.T (h2a cols 0:128, h2b cols 128:256)
WP_W3A = 1280      # [0:128, +0:200]  W3.T[h2 feats 0:128]  (h3a 0:100, h3b 100:200)
WP_W3B = 1480      # [0:128, +0:200]  W3.T[h2 feats 128:256]
WP_WHA = 1680      # [0:102, +0:300]  heads for h3 feats 0:100 + bias hi/lo rows
WP_WHB = 1980      # [0:100, +0:300]  heads for h3 feats 100:200
WP_COLS = 2280


def _split_multi_waits(nc):
    # walrus CoreV3 codegen accepts only one sync-wait per instruction; Tile's
    # exit drain waits once per active proc. Split into single-wait drains.
    for bb in nc.main_func.blocks:
        insts = list(bb.instructions)
        out = []
        changed = False
        for inst in insts:
            si = inst.sync_info
            if si is not None and len(si.on_wait) > 1:
                waits = list(si.on_wait)
                for k, w in enumerate(waits[:-1]):
                    d = mybir.InstDrain(name=f"{inst.name}-sw{k}", ins=[], outs=[])
                    d.engine = inst.engine
                    d.sync_info = bass_rust.SyncInfo(on_wait=[w], on_update=[])
                    nc.register_instruction(d)
                    out.append(d)
                si.on_wait = [waits[-1]]
                changed = True
            out.append(inst)
        if changed:
            bb.instructions = out


def _build_nc():
    nc = bass.Bass(trn_type="TRN2")

    xp = nc.dram_tensor("xp", [96, 2048], F32R, kind="ExternalInput")
    ones2 = nc.dram_tensor("ones2", [2, 512], F32R, kind="ExternalInput")
    wp = nc.dram_tensor("wp", [128, WP_COLS], F32R, kind="ExternalInput")
    bp = nc.dram_tensor("bp", [128, 5], F32, kind="ExternalInput")
    gum = nc.dram_tensor("gum", [128, NT * NSUB * GD], F32, kind="ExternalInput")
    rnd = nc.dram_tensor("rnd", [128, NT * NSUB * D], F32, kind="ExternalInput")
    out_d = nc.dram_tensor("out", [128, NT * NSUB * D], F32, kind="ExternalOutput")

    AX = mybir.AxisListType.X
    OP = mybir.AluOpType
    RELU = mybir.ActivationFunctionType.Relu
    EXP = mybir.ActivationFunctionType.Exp

    from contextlib import ExitStack
    with TileContext(nc) as tc, ExitStack() as ctx:
        const = ctx.enter_context(tc.tile_pool(name="const", bufs=1))
        io = ctx.enter_context(tc.tile_pool(name="io", bufs=2))
        act = ctx.enter_context(tc.tile_pool(name="act", bufs=3))
        samp = ctx.enter_context(tc.tile_pool(name="samp", bufs=3))
        # trunk psum: 4 rotating 1-bank tiles hold h1p/h2ap/h2bp/h3ap/h3bp
        # (5 allocations per step, each drained well before its slot returns)
        pT = ctx.enter_context(tc.tile_pool(name="pT", bufs=4, space="PSUM"))
        # heads psum: one 4-bank tile, freed by the two psum->sbuf copy DMAs
        pHD = ctx.enter_context(tc.tile_pool(name="pHD", bufs=1, space="PSUM"))

        # --- load packed inputs (Act queue: x/weights/biases/rand; SP: gum) ---
        xp_s = const.tile([96, 2048], F32R, tag="xp")
        nc.scalar.dma_start(out=xp_s, in_=xp[:, :])
        wp_s = const.tile([128, WP_COLS], F32R, tag="wp")
        nc.scalar.dma_start(out=wp_s, in_=wp[:, :])
        bp_s = const.tile([128, 5], F32, tag="bp")
        nc.scalar.dma_start(out=bp_s, in_=bp[:, :])
        rnd_s = const.tile([128, NT * NSUB * D], F32, tag="rnd")
        nc.scalar.dma_start(out=rnd_s, in_=rnd[:, :])
        outacc = const.tile([128, NT * NSUB * D], F32, tag="outacc")

        # rotating-state dicts keyed by tile index
        live = {}

        def st_gum(g):
            gum_s = io.tile([128, CH, NSUB, GD], F32, tag="gum")
            nc.sync.dma_start(
                out=gum_s,
                in_=gum[:, g * CH * NSUB * GD:(g + 1) * CH * NSUB * GD]
                .rearrange("p (c s e) -> p c s e", c=CH, s=NSUB))
            live[("gum", g)] = gum_s

        def st_h1(i):
            c2 = i // 2
            xt, xq, xa = c2 // 16, (c2 % 16) // 8, c2 % 8
            xcol = 1024 * xt + (i % 2) * 512
            h1p = pT.tile([128, 512], F32, tag="pt")
            nc.tensor.matmul(h1p,
                             lhsT=wp_s[64 * xq:64 * xq + 32,
                                       WP_W1 + 128 * xa:WP_W1 + 128 * (xa + 1)],
                             rhs=xp_s[64 * xq:64 * xq + 32, xcol:xcol + 512],
                             start=True, stop=True)
            h1s = act.tile([128, 512], F32R, tag="h1")
            nc.scalar.activation(h1s, h1p, func=RELU, bias=bp_s[:, 0:1], scale=1.0)
            live[("h1", i)] = h1s

        def st_h2(i):
            h1s = live.pop(("h1", i))
            h2ap = pT.tile([128, 512], F32, tag="pt")
            nc.tensor.matmul(h2ap, lhsT=wp_s[0:128, WP_W2:WP_W2 + 128],
                             rhs=h1s, start=True, stop=True)
            h2bp = pT.tile([128, 512], F32, tag="pt")
            nc.tensor.matmul(h2bp, lhsT=wp_s[0:128, WP_W2 + 128:WP_W2 + 256],
                             rhs=h1s, start=True, stop=True)
            h2s = act.tile([128, 2, 512], F32R, tag="h2")
            nc.scalar.activation(h2s[:, 0, :], h2ap, func=RELU,
                                 bias=bp_s[:, 1:2], scale=1.0)
            nc.scalar.activation(h2s[:, 1, :], h2bp, func=RELU,
                                 bias=bp_s[:, 2:3], scale=1.0)
            live[("h2", i)] = h2s

        def st_h3(i):
            h2s = live.pop(("h2", i))
            h3ap = pT.tile([128, 512], F32, tag="pt")
            nc.tensor.matmul(h3ap[0:100, :], lhsT=wp_s[0:128, WP_W3A:WP_W3A + 100],
                             rhs=h2s[:, 0, :], start=True, stop=False)
            nc.tensor.matmul(h3ap[0:100, :], lhsT=wp_s[0:128, WP_W3B:WP_W3B + 100],
                             rhs=h2s[:, 1, :], start=False, stop=True)
            h3bp = pT.tile([128, 512], F32, tag="pt")
            nc.tensor.matmul(h3bp[0:100, :], lhsT=wp_s[0:128, WP_W3A + 100:WP_W3A + 200],
                             rhs=h2s[:, 0, :], start=True, stop=False)
            nc.tensor.matmul(h3bp[0:100, :], lhsT=wp_s[0:128, WP_W3B + 100:WP_W3B + 200],
                             rhs=h2s[:, 1, :], start=False, stop=True)
            h3sa = act.tile([102, 512], F32R, tag="h3a")
            if i < 3:
                # ones rows for the head-bias hi/lo trick; each of the 3
                # rotating buffers is initialized once and never clobbered
                # (the relu below only writes rows 0:100). DMA because
                # vector-engine writes must start at partition 0/32/64/96.
                nc.scalar.dma_start(out=h3sa[100:102, :], in_=ones2[:, :])
            nc.scalar.activation(h3sa[0:100, :], h3ap[0:100, :], func=RELU,
                                 bias=bp_s[0:100, 3:4], scale=1.0)
            h3sb = act.tile([100, 512], F32R, tag="h3b")
            nc.scalar.activation(h3sb, h3bp[0:100, :], func=RELU,
                                 bias=bp_s[0:100, 4:5], scale=1.0)
            live[("h3", i)] = (h3sa, h3sb)

        def st_heads(i):
            h3sa, h3sb = live.pop(("h3", i))
            hp = pHD.tile([128, NSUB, 512], F32, tag="hp")
            for s in range(NSUB):
                c0, c1 = s * 128, (s + 1) * 128
                nc.tensor.matmul(hp[:, s, 0:300], lhsT=h3sa[:, c0:c1],
                                 rhs=wp_s[0:102, WP_WHA:WP_WHA + 300],
                                 start=True, stop=False)
                nc.tensor.matmul(hp[:, s, 0:300], lhsT=h3sb[:, c0:c1],
                                 rhs=wp_s[0:100, WP_WHB:WP_WHB + 300],
                                 start=False, stop=True)
            live[("hp", i)] = hp

        EXG = 4          # tiles per batched exp op

        # NOTE: head columns and gumbel are packed D-MAJOR (col = d*25 + g)
        # so every sampling view collapses to <=3D APs (ScalarTensorTensor
        # ISA limit): [p, (s d), g] with (s d) contiguous-nested.

        def st_samp(i):
            hp = live.pop(("hp", i))
            if i % EXG == 0:
                # one exp over EXG tiles' gumbel amortizes the Act bubble
                gum_s = live[("gum", i // CH)]
                ex_g = samp.tile([128, EXG, NSUB, GD], F32, tag="ex")
                j = i % CH
                nc.scalar.activation(
                    ex_g.rearrange("p c s e -> p (c s e)"),
                    gum_s[:, j:j + EXG].rearrange("p c s e -> p (c s e)"),
                    func=EXP)
                live[("ex", i // EXG)] = ex_g
            ex = live[("ex", i // EXG)][:, i % EXG]

            # rnd expanded over g on the (otherwise idle) Pool engine (bf16)
            rnd_e = samp.tile([128, NSUB, GD], BF16, tag="rnde")
            nc.gpsimd.tensor_copy(
                out=rnd_e.rearrange("p s (d g) -> p (s d) g", d=D),
                in_=rnd_s[:, i * 16:(i + 1) * 16]
                .unsqueeze(2).broadcast_to([128, NSUB * D, G]))

            # scs = pai * exp(gumbel)   (signed score; also drains pai psum)
            scs = samp.tile([128, NSUB, GD], F32, tag="scs")
            nc.vector.tensor_tensor(out=scs, in0=hp[:, :, 200:300], in1=ex,
                                    op=OP.mult)
            # ascs = |scs| via the stt (x*-1) max x idiom (all SBUF)
            ascs = samp.tile([128, NSUB, GD], F32, tag="ascs")
            nc.vector.scalar_tensor_tensor(out=ascs, in0=scs, scalar=-1.0,
                                           in1=scs, op0=OP.mult, op1=OP.max)
            # asig = |sig| on Act (drains sig psum), bf16 for the 2x zz mul
            asig = samp.tile([128, NSUB, GD], BF16, tag="asig")
            nc.scalar.activation(asig, hp[:, :, 100:200],
                                 func=mybir.ActivationFunctionType.Abs)
            zz = samp.tile([128, NSUB, GD], BF16, tag="zz")
            nc.vector.tensor_tensor(out=zz, in0=asig, in1=rnd_e, op=OP.mult)
            # z = zz + mu               (drains mu psum)
            z = samp.tile([128, NSUB, GD], BF16, tag="z")
            nc.vector.tensor_tensor(out=z, in0=zz, in1=hp[:, :, 0:100],
                                    op=OP.add)

            # smax = max_g |scs|
            smax = samp.tile([128, NSUB * D], F32, tag="smax")
            nc.vector.tensor_reduce(
                smax, ascs.rearrange("p s (d g) -> p (s d) g", d=D),
                axis=AX, op=OP.max)
            # oh = (|scs| >= smax)
            oh = samp.tile([128, NSUB, GD], BF16, tag="oh")
            nc.vector.tensor_tensor(
                out=oh.rearrange("p s (d g) -> p (s d) g", d=D),
                in0=ascs.rearrange("p s (d g) -> p (s d) g", d=D),
                in1=smax.unsqueeze(2).broadcast_to([128, NSUB * D, G]),
                op=OP.is_ge)
            # zoh = z * oh (bf16 2x), sel = sum_g zoh -> outacc (f32)
            zoh = samp.tile([128, NSUB, GD], BF16, tag="zoh")
            nc.vector.tensor_tensor(out=zoh, in0=z, in1=oh, op=OP.mult)
            nc.vector.tensor_reduce(
                outacc[:, i * 16:(i + 1) * 16],
                zoh.rearrange("p s (d g) -> p (s d) g", d=D),
                axis=AX, op=OP.add)

        # software-pipelined emission: step i runs h1(i), h2(i-1), h3(i-2),
        # heads(i-3), sampling(i-3); gumbel group DMAs prefetch ahead.
        for step in range(NT + 3):
            if step < NT and step % CH == 0:
                st_gum(step // CH)
            if 1 <= step < NT + 1:
                st_h2(step - 1)
            if 2 <= step < NT + 2:
                st_h3(step - 2)
            if step < NT:
                st_h1(step)
            if 3 <= step:
                st_heads(step - 3)
                st_samp(step - 3)

        nc.sync.dma_start(out=out_d[:, :], in_=outacc)

    _split_multi_waits(nc)
    return nc


def _pack_weights(W1, b1, W2, b2, W3, b3, Wmu, bmu, Wsig, bsig, Wpai, bpai):
    # WH: [200, 300] stacked head weights, col = head*100 + d*25 + g (D-MAJOR)
    WH = np.zeros((H3, 300), np.float32)
    bh = np.zeros((300,), np.float32)
    for hd, (W, b) in enumerate([(Wmu, bmu), (Wsig, bsig), (Wpai, bpai)]):
        Wdm = np.asarray(W, np.float32).transpose(1, 0, 2)       # [D, G, H3]
        WH[:, hd * GD:(hd + 1) * GD] = Wdm.reshape(GD, H3).T
        bh[hd * GD:(hd + 1) * GD] = np.asarray(b, np.float32).T.reshape(GD)
    # bias hi/lo split: hi exactly representable at 10 mantissa bits (fp32r)
    bh_hi = (bh.view(np.uint32) & np.uint32(0xFFFFE000)).view(np.float32)
    bh_lo = bh - bh_hi

    wpk = np.zeros((128, WP_COLS), np.float32)
    w1t = np.asarray(W1, np.float32).T            # [3, 128]
    for a in range(8):
        wpk[4 * a:4 * a + 3, WP_W1 + 128 * a:WP_W1 + 128 * (a + 1)] = w1t
        wpk[64 + 4 * a:64 + 4 * a + 3, WP_W1 + 128 * a:WP_W1 + 128 * (a + 1)] = w1t
    wpk[0:128, WP_W2:WP_W2 + 256] = np.asarray(W2, np.float32).T
    w3t = np.asarray(W3, np.float32).T            # [256, 200]
    wpk[0:128, WP_W3A:WP_W3A + 200] = w3t[0:128]
    wpk[0:128, WP_W3B:WP_W3B + 200] = w3t[128:256]
    wpk[0:100, WP_WHA:WP_WHA + 300] = WH[0:100]
    wpk[100, WP_WHA:WP_WHA + 300] = bh_hi
    wpk[101, WP_WHA:WP_WHA + 300] = bh_lo
    wpk[0:100, WP_WHB:WP_WHB + 300] = WH[100:200]

    bpk = np.zeros((128, 5), np.float32)
    bpk[:, 0] = np.asarray(b1, np.float32)
    b2 = np.asarray(b2, np.float32)
    bpk[:, 1] = b2[0:128]
    bpk[:, 2] = b2[128:256]
    b3 = np.asarray(b3, np.float32)
    bpk[0:100, 3] = b3[0:100]
    bpk[0:100, 4] = b3[100:200]
    return np.ascontiguousarray(wpk), np.ascontiguousarray(bpk)


_NC_CACHE = None
LAST_RESULT = None


def kernel(x0, rand, gumbel, W1, b1, W2, b2, W3, b3,
           Wmu, bmu, Wsig, bsig, Wpai, bpai):
    global _NC_CACHE, LAST_RESULT
    if _NC_CACHE is None:
        _NC_CACHE = _build_nc()
    nc = _NC_CACHE

    x0 = np.asarray(x0, np.float32)
    rand = np.asarray(rand, np.float32)
    gumbel = np.asarray(gumbel, np.float32)

    wpk, bpk = _pack_weights(W1, b1, W2, b2, W3, b3,
                             Wmu, bmu, Wsig, bsig, Wpai, bpai)

    in_maps = []
    for c in range(NCORES):
        sl = slice(c * BS, (c + 1) * BS)
        xc = x0[sl]                                   # [32768, 3]
        # chunk c (1024 rows): stripe t=c//16 (cols 1024t:+1024), band
        # q=(c%16)//8 (partitions 64q+...), variant a=c%8 (rows 64q+4a+f)
        xq4 = xc.reshape(2, 2, 8, 1024, 3)            # [t, q, a, jj, f]
        xpk = np.zeros((96, 2048), np.float32)
        for t in range(2):
            for q in range(2):
                blk = xq4[t, q].transpose(0, 2, 1)    # [a, f, jj]
                blk = np.concatenate(
                    [blk, np.zeros((8, 1, 1024), np.float32)], axis=1)
                xpk[64 * q:64 * q + 32, 1024 * t:1024 * (t + 1)] = \
                    blk.reshape(32, 1024)
        # d-major gumbel columns (e = d*25 + g), rows r = 512*it+128*s+p
        gc = gumbel[sl].transpose(0, 2, 1).reshape(BS, GD)
        gpk = np.ascontiguousarray(
            gc.reshape(NT, NSUB, 128, GD).transpose(2, 0, 1, 3)
            .reshape(128, NT * NSUB * GD))
        rc = rand[sl]
        rpk = np.ascontiguousarray(
            rc.reshape(NT, NSUB, 128, D).transpose(2, 0, 1, 3)
            .reshape(128, NT * NSUB * D))
        in_maps.append({"xp": xpk, "wp": wpk, "bp": bpk,
                        "gum": gpk, "rnd": rpk,
                        "ones2": np.ones((2, 512), np.float32)})

    res = run_bass_kernel_spmd(nc, in_maps, core_ids=list(range(NCORES)))
    LAST_RESULT = res
    outs = []
    for c in range(NCORES):
        o = res.results[c]["out"]                     # [128, 1024]
        outs.append(o.reshape(128, NT, NSUB, D).transpose(1, 2, 0, 3)
                    .reshape(BS, D))
    return np.ascontiguousarray(np.concatenate(outs, axis=0).astype(np.float32))
